# revision 14
# baseline (speedup 1.0000x reference)
"""Trainium2 Bass kernel for nn_AttnRelPE (sparse relational attention).

Self-contained: kernel(**inputs) takes FULL inputs, shards policy (dst) rows
and their incident edges across 8 NeuronCores, runs a Bass/Tile kernel via
run_bass_kernel_spmd, returns the FULL [2048, 256] f32 output.

Sharding: core c owns policy rows [256c, 256c+256) and all edges into them
(host-sorted by dst, padded per 128-dst block so each 128-edge tile maps into
one dst block). Segment softmax sums become per-tile one-hot [128x128]
matmuls accumulated in PSUM. No collectives.

Host folds all LN affine params into downstream weights; device LN is a pure
(x-mean)*rstd. Softmax runs without per-segment max subtraction (|sim|<~10
for this model); the k-bias becomes a per-(dst,head) qb gathered with q; the
v-bias is applied dst-side as z*vb. rel-PE fourier LN uses the analytic
variance 0.5-mu^2 (sin/cos pairs) with a Newton rsqrt (no ACT table switch).
"""
import sys

sys.path.insert(0, '/opt/trn_rl_repo')
sys.path.insert(0, '/root/.axon_site')

import numpy as np
import ml_dtypes

D, H, HD, L = 256, 8, 64, 2
NP, NO, NM = 2048, 8192, 16384
NCORES = 8
PC = NP // NCORES          # dst rows per core (256)
CH = 8                     # edge tiles (of 128 edges) per chunk
BF = ml_dtypes.bfloat16

_TWO_PI = float(2.0 * np.pi)
_INV_2PI = float(1.0 / (2.0 * np.pi))
_PI = float(np.pi)


# --------------------------------------------------------------------------
# host-side preparation
# --------------------------------------------------------------------------

def _perm_vector():
    # permuted fourier layout: col p<128 -> sin of orig col i*64+2j;
    # p>=128 -> cos of orig col i*64+2j+1  (i=p//32, j=p%32)
    perm = np.zeros(256, np.int64)
    for i in range(4):
        for j in range(32):
            perm[i * 32 + j] = i * 64 + 2 * j
            perm[128 + i * 32 + j] = i * 64 + 2 * j + 1
    return perm


PERM = _perm_vector()
INV_S = (10000.0 ** (-np.arange(32) / 32.0)).astype(np.float64)


def _prep_edges(edge):
    src, dst = np.asarray(edge[0]), np.asarray(edge[1])
    order = np.argsort(dst, kind='stable')
    src, dst = src[order], dst[order]
    per_core = []
    for c in range(NCORES):
        m = (dst >= PC * c) & (dst < PC * (c + 1))
        s, dl = src[m], (dst[m] - PC * c)
        b0 = dl < 128
        per_core.append(((s[b0], dl[b0]), (s[~b0], dl[~b0] - 128)))
    T0 = max((len(pc[0][0]) + 127) // 128 for pc in per_core)
    T1 = max((len(pc[1][0]) + 127) // 128 for pc in per_core)
    Epad = (T0 + T1) * 128
    srcidx = np.zeros((NCORES, Epad), np.int16)
    dstloc = np.zeros((NCORES, Epad), np.int16)
    valid = np.zeros((NCORES, Epad), bool)
    for c, ((s0, d0), (s1, d1)) in enumerate(per_core):
        n0, n1 = len(s0), len(s1)
        srcidx[c, :n0] = s0
        dstloc[c, :n0] = d0
        valid[c, :n0] = True
        off = T0 * 128
        srcidx[c, off:off + n1] = s1
        dstloc[c, off:off + n1] = d1
        valid[c, off:off + n1] = True
        # padded rows keep dstloc 0 (block-local); _abs_dst maps to valid rows
    return srcidx, dstloc, valid, T0, T1


def _idx_sbuf_layout(idx):
    """[C, E] -> [C, 128, E//16] int16 (wrapped in 16 partitions, replicated)."""
    E = idx.shape[-1]
    t = idx.reshape(idx.shape[0], E // 16, 16).transpose(0, 2, 1)
    return np.ascontiguousarray(np.tile(t, (1, 8, 1)).astype(np.int16))


def _sblk(dstloc, valid, T0):
    C, Epad = dstloc.shape
    out = np.zeros((C, Epad, 128), np.float32)
    for c in range(C):
        col = dstloc[c].astype(np.int64)
        rows = np.arange(Epad)[valid[c]]
        out[c, rows, col[valid[c]]] = 1.0
    return out.astype(BF)


def _posori(pos, ori):
    n = pos.shape[0]
    t = np.zeros((n, 64), np.float32)
    t[:, 0:2] = pos
    t[:, 2] = ori[:, 0]
    return t


def _fold_layer(p):
    g = {k: np.asarray(v, np.float64) for k, v in p.items()}
    ws, bsrc = g['ln_src_w'], g['ln_src_b']
    wd, bd = g['ln_dst_w'], g['ln_dst_b']
    wr, br = g['ln_r_w'][PERM], g['ln_r_b'][PERM]
    Wk = g['Wk'] * ws[:, None]
    bk = g['bk'] + bsrc @ g['Wk']
    Wv = g['Wv'] * ws[:, None]
    bv = g['bv'] + bsrc @ g['Wv']
    Wkr = g['Wkr'][PERM] * wr[:, None]
    bkr = g['bkr'] + br @ g['Wkr'][PERM]
    Wvr = g['Wvr'][PERM] * wr[:, None]
    bvr = g['bvr'] + br @ g['Wvr'][PERM]
    Wq = g['Wq'] * wd[:, None]
    bq = g['bq'] + bd @ g['Wq']
    Wg_ = g['Wg'].copy()
    Wg_[512:] *= wd[:, None]
    bg = g['bg'] + bd @ g['Wg'][512:]
    Ws_ = g['Ws'] * wd[:, None]
    bs2 = g['bs'] + bd @ g['Ws']
    w2, b2w = g['ln2_w'], g['ln2_b']
    W1 = g['W1'] * w2[:, None]
    b1 = g['b1'] + b2w @ g['W1']
    return {
        'Wkcat': np.vstack([Wk, Wkr]),
        'Wvcat': np.vstack([Wv, Wvr]),
        'kb': bk + bkr, 'vb': bv + bvr,
        'Wq': Wq / 8.0, 'bq': bq / 8.0,
        'Wg': Wg_, 'bg': bg, 'Ws': Ws_, 'bs': bs2,
        'Wout': g['Wout'], 'bout': g['bout'],
        'W1': W1, 'b1': b1, 'W2': g['W2'], 'b2': g['b2'],
    }


def _repl(v, n=128):
    v = np.asarray(v, np.float32)
    return np.ascontiguousarray(np.tile(v[None, :], (n, 1)))


# --------------------------------------------------------------------------
# device builder
# --------------------------------------------------------------------------

def _build(nc, Ta0, Ta1, Tm0, Tm1, stage=3, p2sub=5, dbg=False):
    import concourse.bass as bass  # noqa
    import concourse.mybir as mybir
    from concourse.tile import TileContext, add_dep_helper
    from concourse.masks import make_identity

    dt = mybir.dt
    AF = mybir.ActivationFunctionType
    ALU = mybir.AluOpType
    AX = mybir.AxisListType

    Ta, Tm = Ta0 + Ta1, Tm0 + Tm1
    Ea, Em = Ta * 128, Tm * 128

    def din(name, shape, dtype=dt.float32):
        return nc.declare_dram_parameter(name, list(shape), dtype, isOutput=False)

    obs_emd = din("obs_emd", [NO, D])
    map_emd = din("map_emd", [NM, D])
    obs_po = din("obs_po", [NO, 64])
    map_po = din("map_po", [NM, 64])
    pol_po = din("pol_po", [PC, 64])
    x0_ext = din("x0", [PC, D])
    sidx_a = din("sidx_a", [128, Ea // 16], dt.int16)
    didx_a = din("didx_a", [128, Ea // 16], dt.int16)
    sidx_m = din("sidx_m", [128, Em // 16], dt.int16)
    didx_m = din("didx_m", [128, Em // 16], dt.int16)
    sblk_a = din("sblk_a", [Ea, 128], dt.bfloat16)
    sblk_m = din("sblk_m", [Em, 128], dt.bfloat16)
    invs_c = din("invs_c", [128, 128])
    W = {}
    for li in range(4):
        for nm, shape in [
            ('Wkcat', [512, 512]), ('Wvcat', [512, 512]),
            ('Wq', [256, 512]), ('Wg', [768, 512]), ('Ws', [256, 512]),
            ('Wout', [512, 256]), ('W1', [256, 1024]), ('W2', [1024, 256]),
            ('kb', [128, 512]), ('vb', [128, 512]), ('bq', [128, 512]),
            ('bg', [128, 512]), ('bs', [128, 512]), ('bout', [128, 256]),
            ('b1', [128, 1024]), ('b2', [128, 256]),
        ]:
            W[(li, nm)] = din(f"L{li}_{nm}", shape, dt.bfloat16)
    out_ext = nc.declare_dram_parameter("out", [PC, D], dt.float32, isOutput=True)
    DBG = {}
    if dbg:
        DBG['q'] = nc.declare_dram_parameter("dbg_q", [PC, 640], dt.float32, isOutput=True)
        DBG['sim'] = nc.declare_dram_parameter("dbg_sim", [128, CH, 8], dt.float32, isOutput=True)
        DBG['K0'] = nc.declare_dram_parameter("dbg_K0", [128, 512], dt.float32, isOutput=True)
        DBG['ev0'] = nc.declare_dram_parameter("dbg_ev0", [128, 512], dt.float32, isOutput=True)
        DBG['xc'] = nc.declare_dram_parameter("dbg_xc", [128, 4, 1024], dt.float32, isOutput=True)
        DBG['agg'] = nc.declare_dram_parameter("dbg_agg", [128, 512], dt.float32, isOutput=True)
        DBG['z'] = nc.declare_dram_parameter("dbg_z", [128, 8], dt.float32, isOutput=True)

    def reg_const(value, dtype=dt.float32):
        if (dtype, value) not in nc.const_aps.aps:
            t = nc.alloc_sbuf_tensor(f"const-{dtype.name}-{value}", [128, 1], dtype)
            nc.gpsimd.memset(t.ap(), value)
            nc.const_aps.aps[(dtype, value)] = t.ap()

    for v in (1e-5, 1e-20, _PI / 2):
        reg_const(v)

    with TileContext(nc) as tc:
        with tc.tile_pool(name="dram", bufs=1, space="DRAM") as dpool, \
             tc.tile_pool(name="dram2", bufs=2, space="DRAM") as dpool2, \
             tc.tile_pool(name="persist", bufs=1) as pers:

            ln_obs = dpool.tile([NO, D], dt.bfloat16, tag="ln_obs")
            ln_map = dpool.tile([NM, D], dt.bfloat16, tag="ln_map")
            xcat_a = dpool.tile([128, 4, Ea], dt.bfloat16, tag="xcat_a")
            xcat_m = dpool.tile([128, 4, Em], dt.bfloat16, tag="xcat_m")
            rnat_a = dpool.tile([Ea, 256], dt.bfloat16, tag="rnat_a")
            rnat_m = dpool.tile([Em, 256], dt.bfloat16, tag="rnat_m")

            x_sb = [pers.tile([128, D], dt.float32, tag=f"x{b}", name=f"x{b}")
                    for b in range(2)]
            ident = pers.tile([128, 128], dt.bfloat16, tag="ident")
            make_identity(nc, ident[:])
            invs = pers.tile([128, 128], dt.float32, tag="invs")
            nc.sync.dma_start(out=invs[:], in_=invs_c[:])
            sidx_at = pers.tile([128, Ea // 16], dt.int16, tag="sidxa")
            didx_at = pers.tile([128, Ea // 16], dt.int16, tag="didxa")
            sidx_mt = pers.tile([128, Em // 16], dt.int16, tag="sidxm")
            didx_mt = pers.tile([128, Em // 16], dt.int16, tag="didxm")
            for t_, e_ in [(sidx_at, sidx_a), (didx_at, didx_a),
                           (sidx_mt, sidx_m), (didx_mt, didx_m)]:
                nc.sync.dma_start(out=t_[:], in_=e_[:])
            for b in range(2):
                nc.sync.dma_start(out=x_sb[b][:], in_=x0_ext[128 * b:128 * (b + 1), :])

            def ln_normalize(pool, xt, n_free, tagp, out_dt=dt.bfloat16):
                """pure LN of [128, F] f32 -> [128, F] out_dt"""
                st = pool.tile([128, 6], dt.float32, tag=tagp + "st")
                mv = pool.tile([128, 2], dt.float32, tag=tagp + "mv")
                nc.vector.bn_stats(st[:], xt)
                nc.vector.bn_aggr(mv[:], st[:])
                sd = pool.tile([128, 2], dt.float32, tag=tagp + "sd")
                nc.scalar.activation(sd[:, 0:1], mv[:, 1:2], AF.Sqrt, bias=1e-5)
                nc.vector.reciprocal(sd[:, 1:2], sd[:, 0:1])
                o = pool.tile([128, n_free], out_dt, tag=tagp + "out")
                nc.vector.scalar_tensor_tensor(
                    o[:], xt, mv[:, 0:1], sd[:, 1:2].broadcast_to([128, n_free]),
                    op0=ALU.subtract, op1=ALU.mult)
                return o

            # ---------------- phase 1: LN tables ----------------
            ln_writes = {"a": [], "m": []}
            with tc.tile_pool(name="p1", bufs=3) as p1:
                for src_ext, ntab, ltab, lkey in [(obs_emd, NO, ln_obs, "a"),
                                                  (map_emd, NM, ln_map, "m")]:
                    for t in range(ntab // 128):
                        xt = p1.tile([128, D], dt.float32, tag="p1x")
                        nc.sync.dma_start(out=xt[:],
                                          in_=src_ext[128 * t:128 * (t + 1), :])
                        o = ln_normalize(p1, xt[:], D, "p1")
                        wi = nc.sync.dma_start(out=ltab[128 * t:128 * (t + 1), :],
                                               in_=o[:])
                        ln_writes[lkey].append(wi.ins)

            # ---------------- phase 2: per-edge features ----------------
            def phase2(T, sidx_t, didx_t, po_src_ext, ltab, xcat, rnat, lws):
                _ = p2sub
                with tc.tile_pool(name="p2", bufs=2) as p2, \
                     tc.tile_pool(name="p2s", bufs=1) as p2s:
                    feat4 = p2s.tile([128, T, 4], dt.float32, tag="feat4")
                    trig = p2s.tile([128, T, 4], dt.float32, tag="trig")
                    # pass A: geometry (DVE + Sqrt)
                    for c0 in range(0, T, CH):
                        cw = min(CH, T - c0)
                        ps = p2.tile([128, CH, 64], dt.float32, tag="ps")
                        pd = p2.tile([128, CH, 64], dt.float32, tag="pd")
                        nc.gpsimd.dma_gather(
                            out_ap=ps[:, :cw, :], in_ap=po_src_ext[:],
                            idxs_ap=sidx_t[:, c0 * 8:(c0 + cw) * 8],
                            num_idxs=cw * 128, num_idxs_reg=cw * 128, elem_size=64)
                        nc.gpsimd.dma_gather(
                            out_ap=pd[:, :cw, :], in_ap=pol_po[:],
                            idxs_ap=didx_t[:, c0 * 8:(c0 + cw) * 8],
                            num_idxs=cw * 128, num_idxs_reg=cw * 128, elem_size=64)
                        w = p2.tile([128, CH, 8], dt.float32, tag="w")
                        nc.vector.tensor_tensor(w[:, :cw, 0:2], ps[:, :cw, 0:2],
                                                pd[:, :cw, 0:2], op=ALU.subtract)
                        nc.vector.tensor_tensor(w[:, :cw, 2:4], w[:, :cw, 0:2],
                                                w[:, :cw, 0:2], op=ALU.mult)
                        nc.vector.tensor_tensor(w[:, :cw, 4:5], w[:, :cw, 2:3],
                                                w[:, :cw, 3:4], op=ALU.add)
                        r0 = p2.tile([128, CH, 4], dt.float32, tag="r0")
                        nc.scalar.activation(r0[:, :cw, 0:1], w[:, :cw, 4:5],
                                             AF.Sqrt, bias=1e-20)
                        nc.vector.reciprocal(r0[:, :cw, 1:2], r0[:, :cw, 0:1])
                        nc.vector.tensor_tensor(r0[:, :cw, 2:3], w[:, :cw, 4:5],
                                                r0[:, :cw, 1:2], op=ALU.mult)
                        nc.vector.tensor_tensor(r0[:, :cw, 3:4], r0[:, :cw, 0:1],
                                                r0[:, :cw, 2:3], op=ALU.add)
                        nc.vector.tensor_scalar_mul(feat4[:, c0:c0 + cw, 0:1],
                                                    r0[:, :cw, 3:4], 0.5)
                        # w5=rel_ori raw, w6=od, w7=od+pi/2 -> reduce all 3
                        nc.vector.tensor_tensor(w[:, :cw, 5:6], ps[:, :cw, 2:3],
                                                pd[:, :cw, 2:3], op=ALU.subtract)
                        nc.vector.tensor_copy(w[:, :cw, 6:7], pd[:, :cw, 2:3])
                        nc.vector.tensor_scalar_add(w[:, :cw, 7:8], pd[:, :cw, 2:3],
                                                    _PI / 2)
                        ki = p2.tile([128, CH, 3], dt.int32, tag="ki")
                        kf = p2.tile([128, CH, 3], dt.float32, tag="kf")
                        nc.vector.tensor_scalar_mul(kf[:, :cw, :], w[:, :cw, 5:8],
                                                    _INV_2PI)
                        nc.vector.tensor_copy(ki[:, :cw, :], kf[:, :cw, :])
                        nc.vector.tensor_copy(kf[:, :cw, :], ki[:, :cw, :])
                        red = p2.tile([128, CH, 3], dt.float32, tag="red")
                        nc.vector.scalar_tensor_tensor(
                            red[:, :cw, :], kf[:, :cw, :], -_TWO_PI, w[:, :cw, 5:8],
                            op0=ALU.mult, op1=ALU.add)
                        nc.vector.tensor_copy(feat4[:, c0:c0 + cw, 1:2],
                                              red[:, :cw, 0:1])
                        nc.vector.tensor_copy(trig[:, c0:c0 + cw, 0:2],
                                              red[:, :cw, 1:3])
                        nc.vector.tensor_copy(feat4[:, c0:c0 + cw, 2:4],
                                              w[:, :cw, 0:2])  # stash dx,dy

                    if p2sub < 2:
                        return
                    # pass B: sin_od / cos_od (Sin set)
                    sino = p2s.tile([128, T, 2], dt.float32, tag="sino")
                    nc.scalar.activation(sino[:, :, 0:1], trig[:, :, 0:1], AF.Sin)
                    nc.scalar.activation(sino[:, :, 1:2], trig[:, :, 1:2], AF.Sin)
                    # pass C: cross/dot/ratio (DVE); dx,dy in feat4[2:4]
                    crd = p2s.tile([128, T, 4], dt.float32, tag="crd")
                    nc.vector.tensor_tensor(crd[:, :, 0:1], sino[:, :, 1:2],
                                            feat4[:, :, 3:4], op=ALU.mult)
                    nc.vector.tensor_tensor(crd[:, :, 1:2], sino[:, :, 0:1],
                                            feat4[:, :, 2:3], op=ALU.mult)
                    nc.vector.tensor_tensor(crd[:, :, 0:1], crd[:, :, 0:1],
                                            crd[:, :, 1:2], op=ALU.subtract)
                    nc.vector.tensor_tensor(crd[:, :, 1:2], sino[:, :, 1:2],
                                            feat4[:, :, 2:3], op=ALU.mult)
                    nc.vector.tensor_tensor(crd[:, :, 2:3], sino[:, :, 0:1],
                                            feat4[:, :, 3:4], op=ALU.mult)
                    nc.vector.tensor_tensor(crd[:, :, 1:2], crd[:, :, 1:2],
                                            crd[:, :, 2:3], op=ALU.add)
                    nc.vector.reciprocal(crd[:, :, 2:3], crd[:, :, 1:2])
                    nc.vector.tensor_tensor(crd[:, :, 3:4], crd[:, :, 0:1],
                                            crd[:, :, 2:3], op=ALU.mult)
                    # pass D: arctan + quadrant fix
                    at = p2s.tile([128, T, 3], dt.float32, tag="at")
                    nc.scalar.activation(at[:, :, 0:1], crd[:, :, 3:4], AF.Arctan)
                    nc.scalar.activation(at[:, :, 1:2], crd[:, :, 0:1], AF.Sign)
                    msk = p2s.tile([128, T, 1], dt.float32, tag="msk")
                    nc.vector.tensor_scalar(msk[:], crd[:, :, 1:2], 0.0, None,
                                            op0=ALU.is_lt)
                    nc.vector.scalar_tensor_tensor(at[:, :, 2:3], msk[:], _PI,
                                                   at[:, :, 1:2],
                                                   op0=ALU.mult, op1=ALU.mult)
                    nc.vector.tensor_tensor(feat4[:, :, 2:3], at[:, :, 0:1],
                                            at[:, :, 2:3], op=ALU.add)
                    nc.vector.tensor_copy(feat4[:, :, 3:4], feat4[:, :, 2:3])

                    if p2sub < 3:
                        return
                    # pass G: angles, Sin, LN(r) with Newton rsqrt
                    for c0 in range(0, T, CH):
                        cw = min(CH, T - c0)
                        ang = p2.tile([128, CH, 128], dt.float32, tag="ang")
                        fv = feat4[:, c0:c0 + cw, :] \
                            .rearrange("p t (f o) -> p t f o", o=1) \
                            .broadcast_to([128, cw, 4, 32])
                        iv = invs[:, :].rearrange("p (o i j) -> p o i j", o=1, i=4) \
                            .broadcast_to([128, cw, 4, 32])
                        nc.vector.tensor_tensor(
                            ang[:, :cw, :].rearrange("p t (i j) -> p t i j", i=4),
                            fv, iv, op=ALU.mult)
                        k2i = p2.tile([128, CH, 32], dt.int32, tag="k2i")
                        k2f = p2.tile([128, CH, 32], dt.float32, tag="k2f")
                        nc.vector.tensor_scalar_mul(k2f[:, :cw, :], ang[:, :cw, 0:32],
                                                    _INV_2PI)
                        nc.vector.tensor_copy(k2i[:, :cw, :], k2f[:, :cw, :])
                        nc.vector.tensor_copy(k2f[:, :cw, :], k2i[:, :cw, :])
                        nc.vector.scalar_tensor_tensor(
                            ang[:, :cw, 0:32], k2f[:, :cw, :], -_TWO_PI,
                            ang[:, :cw, 0:32], op0=ALU.mult, op1=ALU.add)
                        r = p2.tile([128, CH, 256], dt.bfloat16, tag="r")
                        acc = p2.tile([128, CH, 8], dt.float32, tag="acc")
                        for t in range(cw):
                            nc.scalar.activation(r[:, t, 0:128], ang[:, t, :], AF.Sin,
                                                 accum_out=acc[:, t, 0:1])
                        for t in range(cw):
                            nc.scalar.activation(r[:, t, 128:256], ang[:, t, :],
                                                 AF.Sin, bias=_PI / 2,
                                                 accum_out=acc[:, t, 1:2])
                        # mu = (a0+a1)/256; v = 0.5+eps-mu^2; y=newton rsqrt(v)
                        nc.vector.tensor_tensor(acc[:, :cw, 2:3], acc[:, :cw, 0:1],
                                                acc[:, :cw, 1:2], op=ALU.add)
                        nc.vector.tensor_scalar_mul(acc[:, :cw, 2:3],
                                                    acc[:, :cw, 2:3], 1.0 / 256.0)
                        nc.vector.tensor_tensor(acc[:, :cw, 3:4], acc[:, :cw, 2:3],
                                                acc[:, :cw, 2:3], op=ALU.mult)
                        nc.vector.tensor_scalar(acc[:, :cw, 3:4], acc[:, :cw, 3:4],
                                                -1.0, 0.5 + 1e-5,
                                                op0=ALU.mult, op1=ALU.add)
                        # y0 = 1.414214; 3 Newton steps: y = y*(1.5 - 0.5*v*y^2)
                        nc.vector.tensor_scalar(acc[:, :cw, 4:5], acc[:, :cw, 3:4],
                                                -0.5 * 2.0, 1.5,
                                                op0=ALU.mult, op1=ALU.add)
                        nc.vector.tensor_scalar_mul(acc[:, :cw, 4:5],
                                                    acc[:, :cw, 4:5], 1.4142135)
                        for _ in range(2):
                            nc.vector.tensor_tensor(acc[:, :cw, 5:6],
                                                    acc[:, :cw, 4:5],
                                                    acc[:, :cw, 4:5], op=ALU.mult)
                            nc.vector.tensor_tensor(acc[:, :cw, 5:6],
                                                    acc[:, :cw, 5:6],
                                                    acc[:, :cw, 3:4], op=ALU.mult)
                            nc.vector.tensor_scalar(acc[:, :cw, 5:6],
                                                    acc[:, :cw, 5:6], -0.5, 1.5,
                                                    op0=ALU.mult, op1=ALU.add)
                            nc.vector.tensor_tensor(acc[:, :cw, 4:5],
                                                    acc[:, :cw, 4:5],
                                                    acc[:, :cw, 5:6], op=ALU.mult)
                        rn = p2.tile([128, CH, 256], dt.bfloat16, tag="rn")
                        for t in range(cw):
                            nc.vector.scalar_tensor_tensor(
                                rn[:, t, :], r[:, t, :], acc[:, t, 2:3],
                                acc[:, t, 4:5].broadcast_to([128, 256]),
                                op0=ALU.subtract, op1=ALU.mult)
                        nc.sync.dma_start(
                            out=rnat[c0 * 128:(c0 + cw) * 128, :]
                                .rearrange("(t p) f -> p t f", p=128),
                            in_=rn[:, :cw, :])

                    if p2sub < 4:
                        return
                    # xs_lnT transposed gather -> xcat rows 0..255
                    # (transpose-mode dma_gather caps at 512 idxs per call)
                    for c0 in range(0, T, 4):
                        cw = min(4, T - c0)
                        g = p2.tile([128, 2, cw * 128], dt.bfloat16, tag="g")
                        gi = nc.gpsimd.dma_gather(
                            out_ap=g[:], in_ap=ltab[:],
                            idxs_ap=sidx_t[:, c0 * 8:(c0 + cw) * 8],
                            num_idxs=cw * 128, num_idxs_reg=cw * 128,
                            elem_size=256, transpose=True)
                        for wi_ in lws:
                            add_dep_helper(gi.ins, wi_,
                                           reason="gather reads ln table")
                        for j in range(2):
                            nc.sync.dma_start(
                                out=xcat[:, j, c0 * 128:(c0 + cw) * 128],
                                in_=g[:, j, :])
                    if p2sub < 5:
                        return
                    # rnT via DMA transpose -> xcat rows 256..511
                    for c0 in range(0, T, CH):
                        cw = min(CH, T - c0)
                        for j in range(2):
                            tt_ = p2.tile([128, CH * 128], dt.bfloat16, tag="tt")
                            nc.sync.dma_start(
                                out=tt_[:, :cw * 128],
                                in_=rnat[c0 * 128:(c0 + cw) * 128,
                                         128 * j:128 * (j + 1)],
                                transpose=True)
                            nc.sync.dma_start(
                                out=xcat[:, 2 + j, c0 * 128:(c0 + cw) * 128],
                                in_=tt_[:, :cw * 128])

            if stage >= 2:
                phase2(Ta, sidx_at, didx_at, obs_po, ln_obs, xcat_a, rnat_a,
                        ln_writes["a"])
                phase2(Tm, sidx_mt, didx_mt, map_po, ln_map, xcat_m, rnat_m,
                        ln_writes["m"])

            # ---------------- phase 3: layers ----------------
            def transpose_bf(pool, psp, src_bf, n128, tagp):
                outs = []
                for i in range(n128):
                    pt = psp.tile([128, 128], dt.bfloat16, tag="trps")
                    nc.tensor.transpose(pt[:], src_bf[:, 128 * i:128 * (i + 1)],
                                        ident[:])
                    ot = pool.tile([128, 128], dt.bfloat16, tag=f"{tagp}{i}")
                    nc.vector.tensor_copy(ot[:], pt[:])
                    outs.append(ot)
                return outs

            qtab_hist = {}   # slot -> list of gather insts from previous user
            for li in range(4 if stage >= 3 else 0):
                is_a = (li % 2 == 0)
                T = Ta if is_a else Tm
                T0 = Ta0 if is_a else Tm0
                xcat = xcat_a if is_a else xcat_m
                sblk_e = sblk_a if is_a else sblk_m
                didx_t = didx_at if is_a else didx_mt

                with tc.tile_pool(name=f"wt{li}", bufs=1) as wp, \
                     tc.tile_pool(name=f"qp{li}", bufs=1) as qp:
                    wk = wp.tile([128, 4, 512], dt.bfloat16, tag="wk")
                    wv = wp.tile([128, 4, 512], dt.bfloat16, tag="wv")
                    wq = wp.tile([128, 2, 512], dt.bfloat16, tag="wq")
                    wgt = wp.tile([128, 6, 512], dt.bfloat16, tag="wg")
                    wst = wp.tile([128, 2, 512], dt.bfloat16, tag="wsx")
                    wo = wp.tile([128, 4, 256], dt.bfloat16, tag="wo")
                    w1 = wp.tile([128, 2, 1024], dt.bfloat16, tag="w1")
                    w2 = wp.tile([128, 8, 256], dt.bfloat16, tag="w2")
                    for t_, nm_ in [(wk, 'Wkcat'), (wv, 'Wvcat'), (wq, 'Wq'),
                                    (wgt, 'Wg'), (wst, 'Ws'), (wo, 'Wout'),
                                    (w1, 'W1'), (w2, 'W2')]:
                        nc.sync.dma_start(
                            out=t_[:],
                            in_=W[(li, nm_)][:].rearrange("(k p) n -> p k n", p=128))
                    consts = {}
                    for nm in ['kb', 'vb', 'bq', 'bg', 'bs', 'bout', 'b1', 'b2']:
                        shp = {'bout': 256, 'b2': 256, 'b1': 1024}.get(nm, 512)
                        ct = wp.tile([128, shp], dt.bfloat16, tag=nm, name=nm)
                        nc.sync.dma_start(out=ct[:], in_=W[(li, nm)][:])
                        consts[nm] = ct

                    qtab = dpool2.tile([PC, 640], dt.bfloat16, tag="qtab",
                                       name=f"qtab{li}")
                    qtab_writes = []
                    qex_gathers = []

                    # ---- dst-side: q + qb -> qtab ----
                    xdlT = []
                    with tc.tile_pool(name=f"dqp{li}", bufs=2, space="PSUM") as dqp:
                        for b in range(2):
                            xdl = ln_normalize(qp, x_sb[b][:], D, f"xdl{b}")
                            xdlT.append(transpose_bf(qp, dqp, xdl[:], 2, f"xT{b}_"))
                        for b in range(2):
                            psq = dqp.tile([128, 512], dt.float32, tag="psq")
                            for kt in range(2):
                                nc.tensor.matmul(psq[:], xdlT[b][kt][:], wq[:, kt, :],
                                                 start=(kt == 0), stop=(kt == 1))
                            qsb = qp.tile([128, 640], dt.bfloat16, tag="qsb")
                            nc.vector.tensor_tensor(qsb[:, 0:512], psq[:],
                                                    consts['bq'][:], op=ALU.add)
                            qkb = qp.tile([128, 512], dt.float32, tag="qkb")
                            nc.vector.tensor_tensor(qkb[:], qsb[:, 0:512],
                                                    consts['kb'][:], op=ALU.mult)
                            with nc.allow_low_precision(reason="qb bf16 store"):
                                nc.vector.tensor_reduce(
                                    qsb[:, 512:520],
                                    qkb[:].rearrange("p (h f) -> p h f", h=8),
                                    axis=AX.X, op=ALU.add)
                            nc.vector.memset(qsb[:, 520:640], 0.0)
                            qwi = nc.sync.dma_start(
                                out=qtab[128 * b:128 * (b + 1), :], in_=qsb[:])
                            qtab_writes.append(qwi.ins)
                            for gi_ in qtab_hist.get(li % 2, []):
                                add_dep_helper(qwi.ins, gi_,
                                               reason="qtab WAR vs old gathers")

                    # ---- edge phase ----
                    with tc.tile_pool(name=f"ez{li}", bufs=2, space="PSUM") as eppz:
                        agg_ps = [eppz.tile([128, 512], dt.float32, tag="agg",
                                            name=f"agg{b}") for b in range(2)]
                        zt = eppz.tile([128, 2, 8], dt.float32, tag="z", bufs=1,
                                       name="zt")
                        with tc.tile_pool(name=f"ep{li}", bufs=2) as ep, \
                             tc.tile_pool(name=f"epp{li}", bufs=2, space="PSUM") as epp:
                            for c0 in range(0, T, CH):
                                cw = min(CH, T - c0)
                                xT = ep.tile([128, 4, CH * 128], dt.bfloat16,
                                             tag="xT")
                                for j in range(4):
                                    nc.sync.dma_start(
                                        out=xT[:, j, :cw * 128],
                                        in_=xcat[:, j, c0 * 128:(c0 + cw) * 128])
                                qex = ep.tile([128, CH, 640], dt.bfloat16, tag="qex")
                                qgi = nc.gpsimd.dma_gather(
                                    out_ap=qex[:, :cw, :], in_ap=qtab[:],
                                    idxs_ap=didx_t[:, c0 * 8:(c0 + cw) * 8],
                                    num_idxs=cw * 128, num_idxs_reg=cw * 128,
                                    elem_size=640)
                                qex_gathers.append(qgi.ins)
                                for wi_ in qtab_writes:
                                    add_dep_helper(qgi.ins, wi_,
                                                   reason="qex reads qtab")
                                sbk = ep.tile([128, CH, 128], dt.bfloat16, tag="sbk")
                                nc.sync.dma_start(
                                    out=sbk[:, :cw, :],
                                    in_=sblk_e[c0 * 128:(c0 + cw) * 128, :]
                                        .rearrange("(t p) d -> p t d", p=128))
                                if dbg and li == 0 and c0 == 0:
                                    xcf = ep.tile([128, 4, 1024], dt.float32, tag="xcf")
                                    nc.vector.tensor_copy(xcf[:], xT[:, :, 0:1024])
                                    nc.sync.dma_start(out=DBG['xc'][:], in_=xcf[:])
                                    qxf = ep.tile([128, 2, 640], dt.float32, tag="qxf")
                                    nc.vector.tensor_copy(qxf[:], qex[:, 0:2, :])
                                    nc.sync.dma_start(out=DBG['q'][0:128, :], in_=qxf[:, 0, :])
                                    nc.sync.dma_start(out=DBG['q'][128:256, :], in_=qxf[:, 1, :])
                                sim = ep.tile([128, CH, 8], dt.float32, tag="sim")
                                for t in range(cw):
                                    psK = epp.tile([128, 512], dt.float32, tag="psK")
                                    for kt in range(4):
                                        nc.tensor.matmul(
                                            psK[:], xT[:, kt, t * 128:(t + 1) * 128],
                                            wk[:, kt, :],
                                            start=(kt == 0), stop=(kt == 3))
                                    if dbg and li == 0 and c0 == 0 and t == 0:
                                        kf0 = ep.tile([128, 512], dt.float32, tag="kf0")
                                        nc.vector.tensor_copy(kf0[:], psK[:])
                                        nc.sync.dma_start(out=DBG['K0'][:], in_=kf0[:])
                                    sp = ep.tile([128, 512], dt.bfloat16, tag="sp")
                                    nc.vector.tensor_tensor(sp[:], psK[:],
                                                            qex[:, t, 0:512],
                                                            op=ALU.mult)
                                    nc.vector.tensor_reduce(
                                        sim[:, t, :],
                                        sp[:].rearrange("p (h f) -> p h f", h=8),
                                        axis=AX.X, op=ALU.add)
                                nc.vector.tensor_tensor(sim[:, :cw, :],
                                                        sim[:, :cw, :],
                                                        qex[:, :cw, 512:520],
                                                        op=ALU.add)
                                ebf = ep.tile([128, CH, 8], dt.bfloat16, tag="ebf")
                                nc.scalar.activation(ebf[:, :cw, :], sim[:, :cw, :],
                                                     AF.Exp)
                                if dbg and li == 0 and c0 == 0:
                                    nc.sync.dma_start(out=DBG['sim'][:], in_=sim[:])
                                for t in range(cw):
                                    psV = epp.tile([128, 512], dt.float32, tag="psV")
                                    for kt in range(4):
                                        nc.tensor.matmul(
                                            psV[:], xT[:, kt, t * 128:(t + 1) * 128],
                                            wv[:, kt, :],
                                            start=(kt == 0), stop=(kt == 3))
                                    ev = ep.tile([128, 512], dt.bfloat16, tag="ev")
                                    ebc = ebf[:, t, :] \
                                        .rearrange("p (h o) -> p h o", o=1) \
                                        .broadcast_to([128, 8, 64])
                                    nc.vector.tensor_tensor(
                                        ev[:].rearrange("p (h f) -> p h f", h=8),
                                        psV[:].rearrange("p (h f) -> p h f", h=8),
                                        ebc, op=ALU.mult)
                                    if dbg and li == 0 and c0 == 0 and t == 0:
                                        evf = ep.tile([128, 512], dt.float32, tag="evf")
                                        nc.vector.tensor_copy(evf[:], ev[:])
                                        nc.sync.dma_start(out=DBG['ev0'][:], in_=evf[:])
                                    gt = c0 + t
                                    blk = 0 if gt < T0 else 1
                                    first = (gt == 0) or (gt == T0)
                                    last = (gt == T0 - 1) or (gt == T - 1)
                                    nc.tensor.matmul(agg_ps[blk][:], sbk[:, t, :],
                                                     ev[:], start=first, stop=last)
                                    nc.tensor.matmul(zt[:, blk, :], sbk[:, t, :],
                                                     ebf[:, t, :],
                                                     start=first, stop=last)

                        qtab_hist[li % 2] = qex_gathers
                        # ---- dst epilogue ----
                        with tc.tile_pool(name=f"dd{li}", bufs=1) as dd, \
                             tc.tile_pool(name=f"ddp{li}", bufs=2,
                                          space="PSUM") as ddp:
                            if dbg and li == 0:
                                af0 = dd.tile([128, 512], dt.float32, tag="af0")
                                nc.vector.tensor_copy(af0[:], agg_ps[0][:])
                                nc.sync.dma_start(out=DBG['agg'][:], in_=af0[:])
                                zf0 = dd.tile([128, 8], dt.float32, tag="zf0")
                                nc.vector.tensor_copy(zf0[:], zt[:, 0, :])
                                nc.sync.dma_start(out=DBG['z'][:], in_=zf0[:])
                            for b in range(2):
                                zf = dd.tile([128, 8], dt.float32, tag="zf")
                                nc.vector.tensor_scalar_add(zf[:], zt[:, b, :], 1e-9)
                                rz = dd.tile([128, 8], dt.float32, tag="rz")
                                nc.vector.reciprocal(rz[:], zf[:])
                                zvb = dd.tile([128, 512], dt.float32, tag="zvb")
                                nc.vector.tensor_tensor(
                                    zvb[:].rearrange("p (h f) -> p h f", h=8),
                                    zt[:, b, :].rearrange("p (h o) -> p h o", o=1)
                                        .broadcast_to([128, 8, 64]),
                                    consts['vb'][:]
                                        .rearrange("p (h f) -> p h f", h=8),
                                    op=ALU.mult)
                                agg = dd.tile([128, 512], dt.float32, tag="agg_sb")
                                nc.vector.tensor_tensor(agg[:], agg_ps[b][:],
                                                        zvb[:], op=ALU.add)
                                nc.vector.tensor_tensor(
                                    agg[:].rearrange("p (h f) -> p h f", h=8),
                                    agg[:].rearrange("p (h f) -> p h f", h=8),
                                    rz[:].rearrange("p (h o) -> p h o", o=1)
                                        .broadcast_to([128, 8, 64]),
                                    op=ALU.mult)
                                agb = dd.tile([128, 512], dt.bfloat16, tag="agb")
                                nc.vector.tensor_copy(agb[:], agg[:])
                                aggT = transpose_bf(dd, ddp, agb[:], 4, "aggT")
                                psg = ddp.tile([128, 512], dt.float32, tag="dps",
                                               name="psg")
                                for kt in range(4):
                                    nc.tensor.matmul(psg[:], aggT[kt][:],
                                                     wgt[:, kt, :],
                                                     start=(kt == 0), stop=False)
                                for kt in range(2):
                                    nc.tensor.matmul(psg[:], xdlT[b][kt][:],
                                                     wgt[:, 4 + kt, :],
                                                     start=False, stop=(kt == 1))
                                gv = dd.tile([128, 512], dt.float32, tag="gv")
                                nc.vector.tensor_tensor(gv[:], psg[:],
                                                        consts['bg'][:], op=ALU.add)
                                nc.scalar.activation(gv[:], gv[:], AF.Sigmoid)
                                pss = ddp.tile([128, 512], dt.float32, tag="dps",
                                               name="pss")
                                for kt in range(2):
                                    nc.tensor.matmul(pss[:], xdlT[b][kt][:],
                                                     wst[:, kt, :],
                                                     start=(kt == 0), stop=(kt == 1))
                                sv = dd.tile([128, 512], dt.float32, tag="sv")
                                nc.vector.tensor_tensor(sv[:], pss[:],
                                                        consts['bs'][:], op=ALU.add)
                                nc.vector.tensor_tensor(sv[:], sv[:], agg[:],
                                                        op=ALU.subtract)
                                ov = dd.tile([128, 512], dt.bfloat16, tag="ov")
                                nc.vector.tensor_tensor(ov[:], gv[:], sv[:],
                                                        op=ALU.mult)
                                nc.vector.tensor_tensor(ov[:], ov[:], agg[:],
                                                        op=ALU.add)
                                oT = transpose_bf(dd, ddp, ov[:], 4, "oT")
                                psx = ddp.tile([128, 256], dt.float32, tag="dps2",
                                               bufs=1, name="psx")
                                for kt in range(4):
                                    nc.tensor.matmul(psx[:], oT[kt][:], wo[:, kt, :],
                                                     start=(kt == 0), stop=(kt == 3))
                                nc.vector.tensor_tensor(x_sb[b][:], x_sb[b][:],
                                                        psx[:], op=ALU.add)
                                nc.vector.tensor_tensor(x_sb[b][:], x_sb[b][:],
                                                        consts['bout'][:], op=ALU.add)
                                hn = ln_normalize(dd, x_sb[b][:], D, "hn")
                                hT = transpose_bf(dd, ddp, hn[:], 2, "hT")
                                r1 = dd.tile([128, 1024], dt.bfloat16, tag="r1")
                                for half in range(2):
                                    psf = ddp.tile([128, 512], dt.float32,
                                                   tag="dps", name="psf")
                                    for kt in range(2):
                                        nc.tensor.matmul(
                                            psf[:], hT[kt][:],
                                            w1[:, kt, 512 * half:512 * (half + 1)],
                                            start=(kt == 0), stop=(kt == 1))
                                    fb = dd.tile([128, 512], dt.float32, tag="fb")
                                    nc.vector.tensor_tensor(
                                        fb[:], psf[:],
                                        consts['b1'][:, 512 * half:512 * (half + 1)],
                                        op=ALU.add)
                                    nc.scalar.activation(
                                        r1[:, 512 * half:512 * (half + 1)],
                                        fb[:], AF.Relu)
                                r1T = transpose_bf(dd, ddp, r1[:], 8, "r1T")
                                psx2 = ddp.tile([128, 256], dt.float32, tag="dps2",
                                                bufs=1, name="psx2")
                                for kt in range(8):
                                    nc.tensor.matmul(psx2[:], r1T[kt][:],
                                                     w2[:, kt, :],
                                                     start=(kt == 0), stop=(kt == 7))
                                nc.vector.tensor_tensor(x_sb[b][:], x_sb[b][:],
                                                        psx2[:], op=ALU.add)
                                nc.vector.tensor_tensor(x_sb[b][:], x_sb[b][:],
                                                        consts['b2'][:], op=ALU.add)

            for b in range(2):
                nc.sync.dma_start(out=out_ext[128 * b:128 * (b + 1), :],
                                  in_=x_sb[b][:])

    return nc


# --------------------------------------------------------------------------
# entry point
# --------------------------------------------------------------------------

_CACHE = {}
_LAST_IN_MAPS = None
_LAST_RES = None


def kernel(policy_emd, policy_pos, policy_ori, obs_emd, obs_pos, obs_ori,
           map_emd, map_pos, map_ori, a2p_edge_index, m2p_edge_index, params):
    from concourse import bacc
    from concourse.bass_utils import run_bass_kernel_spmd

    policy_emd = np.asarray(policy_emd, np.float32)

    sa, da, va, Ta0, Ta1 = _prep_edges(np.asarray(a2p_edge_index))
    sm, dm, vm, Tm0, Tm1 = _prep_edges(np.asarray(m2p_edge_index))
    sblk_a = _sblk(da, va, Ta0)
    sblk_m = _sblk(dm, vm, Tm0)
    sa_l, da_l = _idx_sbuf_layout(sa), _idx_sbuf_layout(_abs_dst(da, Ta0))
    sm_l, dm_l = _idx_sbuf_layout(sm), _idx_sbuf_layout(_abs_dst(dm, Tm0))

    obs_po = _posori(np.asarray(obs_pos, np.float32),
                     np.asarray(obs_ori, np.float32))
    map_po = _posori(np.asarray(map_pos, np.float32),
                     np.asarray(map_ori, np.float32))
    pol_po = _posori(np.asarray(policy_pos, np.float32),
                     np.asarray(policy_ori, np.float32))

    invs_c = np.zeros((128, 128), np.float32)
    for i in range(4):
        for j in range(32):
            invs_c[:, i * 32 + j] = INV_S[j]

    layer_order = [('a2p', 0), ('m2p', 0), ('a2p', 1), ('m2p', 1)]
    folded = [_fold_layer(params[t][i]) for (t, i) in layer_order]

    import os
    stage = int(os.environ.get("K_STAGE", "3"))
    p2sub = int(os.environ.get("K_P2SUB", "5"))
    dbg = bool(int(os.environ.get("K_DEBUG", "0")))
    key = (Ta0, Ta1, Tm0, Tm1, stage, p2sub, dbg)
    if key not in _CACHE:
        nc = bacc.Bacc()
        _build(nc, Ta0, Ta1, Tm0, Tm1, stage=stage, p2sub=p2sub, dbg=dbg)
        nc.finalize()
        _CACHE[key] = nc
    nc = _CACHE[key]

    shared = {
        "obs_emd": np.asarray(obs_emd, np.float32),
        "map_emd": np.asarray(map_emd, np.float32),
        "obs_po": obs_po, "map_po": map_po, "invs_c": invs_c,
    }
    for li, f in enumerate(folded):
        for nm in ['Wkcat', 'Wvcat', 'Wq', 'Wg', 'Ws', 'Wout', 'W1', 'W2']:
            shared[f"L{li}_{nm}"] = f[nm].astype(BF)
        for nm in ['kb', 'vb', 'bq', 'bg', 'bs', 'bout', 'b1', 'b2']:
            shared[f"L{li}_{nm}"] = _repl(f[nm]).astype(BF)

    in_maps = []
    for c in range(NCORES):
        m = dict(shared)
        m.update({
            "pol_po": pol_po[PC * c:PC * (c + 1)],
            "x0": policy_emd[PC * c:PC * (c + 1)],
            "sidx_a": sa_l[c], "didx_a": da_l[c],
            "sidx_m": sm_l[c], "didx_m": dm_l[c],
            "sblk_a": sblk_a[c], "sblk_m": sblk_m[c],
        })
        in_maps.append(m)

    global _LAST_IN_MAPS
    _LAST_IN_MAPS = in_maps
    res = run_bass_kernel_spmd(nc, in_maps, list(range(NCORES)))
    global _LAST_RES
    _LAST_RES = res
    out = np.concatenate([res.results[c]["out"] for c in range(NCORES)], axis=0)
    return out.astype(np.float32)


def _abs_dst(dstloc, T0):
    """dstloc stores block-local 0..127 in both blocks; gathers (qtab,
    pol_po) need absolute 0..255 rows."""
    out = dstloc.astype(np.int32).copy()
    out[:, T0 * 128:] += 128
    return np.clip(out, 0, PC - 1).astype(np.int16)
